# revision 5
# baseline (speedup 1.0000x reference)
"""Trainium2 Bass kernel for nn_NeuralAttention (dense transformer block:
QKV projection + RoPE + softmax attention + output projection).

Sharding: 8 heads -> 8 NeuronCores (tensor parallel, Megatron-style).
Each core computes one head end-to-end from the full input x and produces a
partial output y_h = softmax((q_h k_h^T)/8) v_h @ wo[:, h].T of shape
[4096, 512]; the host sums the 8 partials.

Device-side per core (head h), everything in float32r (tf32-like matmul
dtype, ~1e-4 relative error, full-rate on the PE for moving dim >= 256):

  xT  [512, 4096]  (host-transposed x)
  qcat = x @ [wq_h | P wq_h].T   -> psum [128, 512] per 512-block
         (concat-M matmul: rows 0-63 = q.T, 64-127 = rot(q).T)
  RoPE: tmp = psum * [cosT; sinT]  (DVE), fold: q'T = Istack.T @ tmp (PE)
  S.T chunk [128(tk), q] = kT_chunk.T @ q'T   (K=64)
  A.T = exp(S.T / 8)               (ACT, PSUM -> SBUF f32r)
  O_aug.T [65, q] += v_aug_chunk.T @ A.T chunk   (K=128, accumulated in PSUM;
         row 64 = ones column of v_aug = softmax denominators)
  y = (O.T_chunk.T @ woT) * recip_sums_per_row   -> DMA out [4096, 512]
"""

import numpy as np

import concourse.bacc as bacc
import concourse.tile as tile
from concourse import mybir
from concourse.bass import ds, ts
from concourse.bass_utils import run_bass_kernel_spmd

F32 = mybir.dt.float32
F32R = mybir.dt.float32r
EXP = mybir.ActivationFunctionType.Exp

T = 4096
HIDDEN = 512
N_HEADS = 8
HD = 64
N_CORES = 8
NBLK = T // 512  # 8 column blocks of 512
ROPE_BASE = 10000.0

_CACHE = {}


def _build():
    nc = bacc.Bacc("TRN2", target_bir_lowering=False, debug=False,
                   num_devices=N_CORES)

    xT_d = nc.dram_tensor("xT", [HIDDEN, T], F32, kind="ExternalInput").ap()
    cs_d = nc.dram_tensor("cs", [128, T], F32, kind="ExternalInput").ap()
    wq_d = nc.dram_tensor("wqcat", [HIDDEN, 128], F32, kind="ExternalInput").ap()
    wk_d = nc.dram_tensor("wkcat", [HIDDEN, 128], F32, kind="ExternalInput").ap()
    wv_d = nc.dram_tensor("wvT", [HIDDEN, HD], F32, kind="ExternalInput").ap()
    wo_d = nc.dram_tensor("woT", [HD, HIDDEN], F32, kind="ExternalInput").ap()
    istk_d = nc.dram_tensor("istk", [128, HD], F32, kind="ExternalInput").ap()
    ones_d = nc.dram_tensor("ones", [128, 32], F32, kind="ExternalInput").ap()
    iden_d = nc.dram_tensor("iden", [HD, HD], F32, kind="ExternalInput").ap()
    y_d = nc.dram_tensor("y", [T, HIDDEN], F32, kind="ExternalOutput").ap()

    with tile.TileContext(nc) as tc:
        with tc.tile_pool(name="persist", bufs=1) as sb:
            # persistent SBUF tensors
            xT = sb.tile([128, 4, T], F32R)        # 64 KB/part
            cs = sb.tile([128, T], F32)            # 16 KB/part
            wq = sb.tile([128, 4, 128], F32R)
            wk = sb.tile([128, 4, 128], F32R)
            wv = sb.tile([128, 4, HD], F32R)
            wo = sb.tile([HD, HIDDEN], F32R)
            istk = sb.tile([128, HD], F32R)
            iden = sb.tile([HD, HD], F32R)
            qT = sb.tile([HD, T], F32R)            # 16 KB/part
            kT = sb.tile([HD, T], F32R)
            va = sb.tile([128, 32, HD + 1], F32R)  # v_aug chunks, 8.3 KB/part
            OT = sb.tile([HD, T], F32R)            # 16 KB/part
            sumrow = sb.tile([1, T], F32)
            recipT = sb.tile([128, 32], F32)

            # input DMAs (xT per (k, block) for pipelining with projections)
            xT_r = xT_d.rearrange("(c p) t -> p c t", p=128)
            for k in range(4):
                for b in range(NBLK):
                    nc.sync.dma_start(xT[:, k, ts(b, 512)],
                                      xT_r[:, k, ts(b, 512)].bitcast(F32R))
            nc.sync.dma_start(cs[:], cs_d)
            nc.sync.dma_start(wq[:], wq_d.rearrange("(c p) m -> p c m", p=128).bitcast(F32R))
            nc.sync.dma_start(wk[:], wk_d.rearrange("(c p) m -> p c m", p=128).bitcast(F32R))
            nc.sync.dma_start(wv[:], wv_d.rearrange("(c p) m -> p c m", p=128).bitcast(F32R))
            nc.sync.dma_start(wo[:], wo_d.bitcast(F32R))
            nc.sync.dma_start(istk[:], istk_d.bitcast(F32R))
            nc.sync.dma_start(iden[:], iden_d.bitcast(F32R))

            # ─── Phase P: projections + RoPE + v transposes ───
            with tc.tile_pool(name="pp", bufs=2, space="PSUM") as pp, \
                 tc.tile_pool(name="pf", bufs=2, space="PSUM") as pf, \
                 tc.tile_pool(name="pv", bufs=2, space="PSUM") as pv, \
                 tc.tile_pool(name="ptr", bufs=2, space="PSUM") as ptr, \
                 tc.tile_pool(name="ptmp", bufs=3) as ptmp:

                # warm the exp table set early (overlaps with projections)
                warm = ptmp.tile([1, 16], F32, tag="warm", bufs=1)
                nc.vector.memset(warm[:], 0.0)
                nc.scalar.activation(warm[:], warm[:], EXP, scale=1.0)

                vT = ptmp.tile([HD, T], F32R, tag="vT", bufs=1)  # scratch, freed at pool exit
                pending = []  # (tmp_q, tmp_k, b) awaiting fold
                for b in range(NBLK):
                    pq = pp.tile([128, 512], F32, tag="pp")
                    for k in range(4):
                        nc.tensor.matmul(pq[:], wq[:, k, :], xT[:, k, ts(b, 512)],
                                         start=(k == 0), stop=(k == 3))
                    pk = pp.tile([128, 512], F32, tag="pp")
                    for k in range(4):
                        nc.tensor.matmul(pk[:], wk[:, k, :], xT[:, k, ts(b, 512)],
                                         start=(k == 0), stop=(k == 3))
                    pv_ = pv.tile([HD, 512], F32, tag="pv")
                    for k in range(4):
                        nc.tensor.matmul(pv_[:], wv[:, k, :], xT[:, k, ts(b, 512)],
                                         start=(k == 0), stop=(k == 3))
                    nc.scalar.copy(vT[:, ts(b, 512)], pv_[:].bitcast(F32))

                    tq = ptmp.tile([128, 512], F32R, tag="tq")
                    nc.vector.tensor_tensor(tq[:], pq[:], cs[:, ts(b, 512)],
                                            op=mybir.AluOpType.mult)
                    tk_ = ptmp.tile([128, 512], F32R, tag="tk")
                    nc.vector.tensor_tensor(tk_[:], pk[:], cs[:, ts(b, 512)],
                                            op=mybir.AluOpType.mult)
                    pending.append((tq, tk_, b))
                    if b > 0:  # fold previous block (keeps PE fed while DVE works)
                        tq0, tk0, b0 = pending.pop(0)
                        pfq = pf.tile([HD, 512], F32, tag="pf")
                        nc.tensor.matmul(pfq[:], istk[:], tq0[:], start=True, stop=True)
                        nc.scalar.copy(qT[:, ts(b0, 512)], pfq[:].bitcast(F32))
                        pfk = pf.tile([HD, 512], F32, tag="pf")
                        nc.tensor.matmul(pfk[:], istk[:], tk0[:], start=True, stop=True)
                        nc.scalar.copy(kT[:, ts(b0, 512)], pfk[:].bitcast(F32))
                tq0, tk0, b0 = pending.pop(0)
                pfq = pf.tile([HD, 512], F32, tag="pf")
                nc.tensor.matmul(pfq[:], istk[:], tq0[:], start=True, stop=True)
                nc.scalar.copy(qT[:, ts(b0, 512)], pfq[:].bitcast(F32))
                pfk = pf.tile([HD, 512], F32, tag="pf")
                nc.tensor.matmul(pfk[:], istk[:], tk0[:], start=True, stop=True)
                nc.scalar.copy(kT[:, ts(b0, 512)], pfk[:].bitcast(F32))

                # v transposes: vT [64, T] -> va chunks [128, 64]
                for c in range(32):
                    pt = ptr.tile([128, HD], F32R, tag="ptr")
                    nc.tensor.transpose(pt[:], vT[:, ts(c, 128)], iden[:])
                    nc.vector.tensor_copy(va[:, c, 0:HD], pt[:].bitcast(F32))
                nc.sync.dma_start(va[:, :, HD:HD + 1], ones_d.unsqueeze(2).bitcast(F32R))

            # ─── Phase A: attention ───
            with tc.tile_pool(name="po", bufs=1, space="PSUM") as po, \
                 tc.tile_pool(name="psc", bufs=2, space="PSUM") as psc, \
                 tc.tile_pool(name="pa", bufs=3) as pa:
                for h in range(2):
                    O_ps = po.tile([128, 4, 512], F32, tag="O")
                    prev = None  # A tiles of previous chunk awaiting O-mms
                    for c in range(32):
                        cur = []
                        for half in range(2):
                            s_t = psc.tile([128, 1024], F32, tag="s")
                            for j in range(2):
                                q0 = h * 2048 + half * 1024 + j * 512
                                nc.tensor.matmul(s_t[:, ts(j, 512)],
                                                 kT[:, ts(c, 128)],
                                                 qT[:, q0:q0 + 512],
                                                 start=True, stop=True)
                            a_t = pa.tile([128, 1024], F32R, tag="a")
                            nc.scalar.activation(a_t[:], s_t[:], EXP, scale=0.125)
                            cur.append(a_t)
                        if prev is not None:
                            pc, a_pair = prev
                            for qb in range(4):
                                nc.tensor.matmul(O_ps[0:HD + 1, qb, :],
                                                 va[:, pc, :],
                                                 a_pair[qb // 2][:, ts(qb % 2, 512)],
                                                 start=(pc == 0), stop=(pc == 31))
                        prev = (c, cur)
                    pc, a_pair = prev
                    for qb in range(4):
                        nc.tensor.matmul(O_ps[0:HD + 1, qb, :], va[:, pc, :],
                                         a_pair[qb // 2][:, ts(qb % 2, 512)],
                                         start=(pc == 0), stop=(pc == 31))
                    for qb in range(4):
                        nc.vector.tensor_copy(OT[:, ds(h * 2048 + qb * 512, 512)],
                                              O_ps[0:HD, qb, :])
                        nc.scalar.copy(sumrow[:, ds(h * 2048 + qb * 512, 512)],
                                       O_ps[HD:HD + 1, qb, :])

            # ─── Phase S: softmax denominators -> per-partition reciprocals ───
            with tc.tile_pool(name="dr", bufs=1, space="DRAM") as dr:
                scratch = dr.tile([1, T], F32)
                nc.sync.dma_start(scratch[:], sumrow[:])
                nc.sync.dma_start(
                    recipT[:],
                    scratch[0:1, :].rearrange("x (j p) -> (x p) j", p=128))
            nc.vector.reciprocal(recipT[:], recipT[:])

            # ─── Phase Y: output projection + row normalization ───
            with tc.tile_pool(name="py", bufs=4, space="PSUM") as py, \
                 tc.tile_pool(name="yt", bufs=4) as yt:
                for qc in range(32):
                    p = py.tile([128, 512], F32, tag="y")
                    nc.tensor.matmul(p[:], OT[:, ts(qc, 128)], wo[:],
                                     start=True, stop=True)
                    y_t = yt.tile([128, 512], F32, tag="yt")
                    nc.vector.tensor_scalar_mul(y_t[:], p[:], recipT[:, qc:qc + 1])
                    nc.sync.dma_start(y_d[ts(qc, 128), :], y_t[:])

    nc.compile()
    return nc


def _host_prep(x, wq, wk, wv, wo, timestamp):
    x2 = np.asarray(x, dtype=np.float32).reshape(T, HIDDEN)
    xT = np.ascontiguousarray(x2.T)

    tsamp = np.asarray(timestamp).reshape(T)
    inv = (1.0 / (np.float32(ROPE_BASE)
                  ** (np.arange(0, HD, 2, dtype=np.float32) / np.float32(HD))))
    freqs = tsamp.astype(np.float32)[:, None] * inv[None, :].astype(np.float32)
    emb = np.concatenate([freqs, freqs], axis=1)          # [T, 64]
    cs = np.concatenate([np.cos(emb).T, np.sin(emb).T], axis=0)  # [128, T]
    cs = np.ascontiguousarray(cs, dtype=np.float32)

    P = np.zeros((HD, HD), dtype=np.float32)
    P[np.arange(32), np.arange(32) + 32] = -1.0
    P[np.arange(32) + 32, np.arange(32)] = 1.0

    istk = np.vstack([np.eye(HD), np.eye(HD)]).astype(np.float32)
    iden = np.eye(HD, dtype=np.float32)

    wq = np.asarray(wq, dtype=np.float32)
    wk = np.asarray(wk, dtype=np.float32)
    wv = np.asarray(wv, dtype=np.float32)
    wo = np.asarray(wo, dtype=np.float32)

    in_maps = []
    for h in range(N_HEADS):
        sl = slice(h * HD, (h + 1) * HD)
        wq_h, wk_h, wv_h = wq[sl, :], wk[sl, :], wv[sl, :]
        in_maps.append({
            "xT": xT,
            "cs": cs,
            "wqcat": np.ascontiguousarray(
                np.concatenate([wq_h.T, (P @ wq_h).T], axis=1)),
            "wkcat": np.ascontiguousarray(
                np.concatenate([wk_h.T, (P @ wk_h).T], axis=1)),
            "wvT": np.ascontiguousarray(wv_h.T),
            "woT": np.ascontiguousarray(wo[:, sl].T),
            "istk": istk,
            "ones": np.ones((128, 32), dtype=np.float32),
            "iden": iden,
        })
    return in_maps


def kernel(x, wq, wk, wv, wo, timestamp):
    if "nc" not in _CACHE:
        _CACHE["nc"] = _build()
    nc = _CACHE["nc"]
    in_maps = _host_prep(x, wq, wk, wv, wo, timestamp)
    r = run_bass_kernel_spmd(nc, in_maps, list(range(N_CORES)))
    y = np.zeros((T, HIDDEN), dtype=np.float64)
    for c in range(N_CORES):
        y += r.results[c]["y"].astype(np.float64)
    return y.astype(np.float32).reshape(1, T, HIDDEN)


# revision 7
# speedup vs baseline: 1.5406x; 1.5406x over previous
"""Trainium2 Bass kernel for nn_NeuralAttention (dense transformer block:
QKV projection + RoPE + softmax attention + output projection).

Sharding: 8 heads -> 8 NeuronCores (tensor parallel, Megatron-style).
Each core computes one head end-to-end from the full input x and produces a
partial output y_h = softmax((q_h k_h^T)/8) v_h @ wo[:, h].T of shape
[4096, 512]; the host sums the 8 partials.

All matmul operands are float32r (tf32-like, ~1e-4 relative error, 1 col/cycle
on the PE). Every contraction is padded to K=128: the TRN2 HAM clock gate
only counts the PE "busy" when the full array is active, so K=64 matmuls run
at 1.2 GHz forever while K=128 runs at 2.4 GHz. Padding the contraction with
zeros (producer side) or multiplying garbage rows by zero weights (consumer
side) doubles the clock at zero cycle cost.

Per-core pipeline (head h):
  xT [512, 4096] (host-transposed x) -> q|rot(q) and k|rot(k) concat-M
  projections -> RoPE via DVE mult with [cos;sin] + PE fold with
  [I;I | 0] -> q'T/k'T [128, 4096] (rows 64-127 zero)
  S.T chunk [128(tk), 512(q)] = kT2_chunk.T @ qT2 (K=128, hi half zeros)
  A.T = exp(S.T/8)  (ACT, PSUM->SBUF, f32r)
  O.T [128, q] += va2_chunk.T @ A.T  (va2 cols 64-127 = 1.0 -> rows 64-127 of
  O.T = softmax denominators), accumulated in PSUM over 32 chunks
  y qchunk [128, 512] = (OT2_chunk.T @ wo2) * recip(sums)  (wo2 rows 64-127
  zero), overlapped with the next q-quarter's attention.
"""

import numpy as np

import concourse.bacc as bacc
import concourse.tile as tile
from concourse import mybir
from concourse.bass import ds, ts
from concourse.bass_utils import run_bass_kernel_spmd

F32 = mybir.dt.float32
F32R = mybir.dt.float32r
EXP = mybir.ActivationFunctionType.Exp

T = 4096
HIDDEN = 512
N_HEADS = 8
HD = 64
N_CORES = 8
NBLK = T // 512
ROPE_BASE = 10000.0

_CACHE = {}


def _fold(nc, pf, istk2, qT2, kT2, pending):
    tq0, tk0, b0 = pending.pop(0)
    pfq = pf.tile([128, 512], F32, tag="pf", name="pfq")
    nc.tensor.matmul(pfq[:], istk2[:], tq0[:], start=True, stop=True)
    nc.scalar.copy(qT2[:, ts(b0, 512)], pfq[:].bitcast(F32))
    pfk = pf.tile([128, 512], F32, tag="pf", name="pfk")
    nc.tensor.matmul(pfk[:], istk2[:], tk0[:], start=True, stop=True)
    nc.scalar.copy(kT2[:, ts(b0, 512)], pfk[:].bitcast(F32))


def _build():
    nc = bacc.Bacc("TRN2", target_bir_lowering=False, debug=False,
                   num_devices=N_CORES)

    xT_d = nc.dram_tensor("xT", [HIDDEN, T], F32, kind="ExternalInput").ap()
    cs_d = nc.dram_tensor("cs", [128, T], F32, kind="ExternalInput").ap()
    wq_d = nc.dram_tensor("wqcat", [HIDDEN, 128], F32, kind="ExternalInput").ap()
    wk_d = nc.dram_tensor("wkcat", [HIDDEN, 128], F32, kind="ExternalInput").ap()
    wv_d = nc.dram_tensor("wvT", [HIDDEN, HD], F32, kind="ExternalInput").ap()
    wo_d = nc.dram_tensor("wo2", [128, HIDDEN], F32, kind="ExternalInput").ap()
    istk_d = nc.dram_tensor("istk2", [128, 128], F32, kind="ExternalInput").ap()
    iden_d = nc.dram_tensor("iden", [HD, HD], F32, kind="ExternalInput").ap()
    ones_d = nc.dram_tensor("ones", [128, 32, HD], F32, kind="ExternalInput").ap()
    y_d = nc.dram_tensor("y", [T, HIDDEN], F32, kind="ExternalOutput").ap()

    with tile.TileContext(nc) as tc:
        with tc.tile_pool(name="persist", bufs=1) as sb:
            xT = sb.tile([128, 4, T], F32R)        # 64 KB/part
            cs = sb.tile([128, T], F32)            # 16 KB/part
            wq = sb.tile([128, 4, 128], F32R)
            wk = sb.tile([128, 4, 128], F32R)
            wv = sb.tile([128, 4, HD], F32R)
            wo2 = sb.tile([128, HIDDEN], F32R)     # rows 64-127 zero
            istk2 = sb.tile([128, 128], F32R)      # [[I;I] | 0]
            iden = sb.tile([HD, HD], F32R)
            qT2 = sb.tile([128, T], F32R)          # rows 64-127 zero
            kT2 = sb.tile([128, T], F32R)          # rows 64-127 zero
            va2 = sb.tile([128, 32, 128], F32R)    # cols 64-127 = 1.0
            OT2 = sb.tile([128, T], F32R)          # rows 64-127 = denominators
            recipT = sb.tile([128, 32], F32)

            # input DMAs; xT pieces spread across sync + gpsimd queues
            xT_r = xT_d.rearrange("(c p) t -> p c t", p=128)
            for k in range(4):
                for b in range(NBLK):
                    eng = nc.sync if (b % 2 == 0) else nc.gpsimd
                    eng.dma_start(xT[:, k, ts(b, 512)],
                                  xT_r[:, k, ts(b, 512)].bitcast(F32R))
            nc.sync.dma_start(cs[:], cs_d)
            nc.sync.dma_start(wq[:], wq_d.rearrange("(c p) m -> p c m", p=128).bitcast(F32R))
            nc.sync.dma_start(wk[:], wk_d.rearrange("(c p) m -> p c m", p=128).bitcast(F32R))
            nc.sync.dma_start(wv[:], wv_d.rearrange("(c p) m -> p c m", p=128).bitcast(F32R))
            nc.sync.dma_start(wo2[:], wo_d.bitcast(F32R))
            nc.sync.dma_start(istk2[:], istk_d.bitcast(F32R))
            nc.sync.dma_start(iden[:], iden_d.bitcast(F32R))
            nc.gpsimd.dma_start(va2[:, :, HD:128], ones_d.bitcast(F32R))

            # ─── Phase P: projections + RoPE + v transposes ───
            with tc.tile_pool(name="pp", bufs=2, space="PSUM") as pp, \
                 tc.tile_pool(name="pf", bufs=2, space="PSUM") as pf, \
                 tc.tile_pool(name="pv", bufs=2, space="PSUM") as pv, \
                 tc.tile_pool(name="ptr", bufs=2, space="PSUM") as ptr, \
                 tc.tile_pool(name="ptmp", bufs=3) as ptmp:

                # warm the exp table set early (overlaps with projections)
                warm = ptmp.tile([1, 16], F32, tag="warm", bufs=1)
                nc.vector.memset(warm[:], 0.0)
                nc.scalar.activation(warm[:], warm[:], EXP, scale=1.0)

                vT = ptmp.tile([HD, T], F32R, tag="vT", bufs=1)
                pending = []
                for b in range(NBLK):
                    pq = pp.tile([128, 512], F32, tag="pp", name="pq")
                    for k in range(4):
                        nc.tensor.matmul(pq[:], wq[:, k, :], xT[:, k, ts(b, 512)],
                                         start=(k == 0), stop=(k == 3))
                    pk = pp.tile([128, 512], F32, tag="pp", name="pk")
                    for k in range(4):
                        nc.tensor.matmul(pk[:], wk[:, k, :], xT[:, k, ts(b, 512)],
                                         start=(k == 0), stop=(k == 3))
                    pv_ = pv.tile([HD, 512], F32, tag="pv", name="pv_")
                    for k in range(4):
                        nc.tensor.matmul(pv_[:], wv[:, k, :], xT[:, k, ts(b, 512)],
                                         start=(k == 0), stop=(k == 3))
                    nc.scalar.copy(vT[:, ts(b, 512)], pv_[:].bitcast(F32))

                    tq = ptmp.tile([128, 512], F32R, tag="tq", name="tq")
                    nc.vector.tensor_tensor(tq[:], pq[:], cs[:, ts(b, 512)],
                                            op=mybir.AluOpType.mult)
                    tk_ = ptmp.tile([128, 512], F32R, tag="tk", name="tk_")
                    nc.vector.tensor_tensor(tk_[:], pk[:], cs[:, ts(b, 512)],
                                            op=mybir.AluOpType.mult)
                    pending.append((tq, tk_, b))
                    if b > 0:
                        _fold(nc, pf, istk2, qT2, kT2, pending)
                _fold(nc, pf, istk2, qT2, kT2, pending)

                for c in range(32):
                    pt = ptr.tile([128, HD], F32R, tag="ptr", name="pt")
                    nc.tensor.transpose(pt[:], vT[:, ts(c, 128)], iden[:])
                    nc.vector.tensor_copy(va2[:, c, 0:HD], pt[:].bitcast(F32))

            # ─── Phase A+Y: attention quarters with overlapped output proj ───
            with tc.tile_pool(name="po", bufs=2, space="PSUM") as po, \
                 tc.tile_pool(name="psc", bufs=2, space="PSUM") as psc, \
                 tc.tile_pool(name="pa", bufs=3) as pa, \
                 tc.tile_pool(name="yt", bufs=4) as yt, \
                 tc.tile_pool(name="dr", bufs=4, space="DRAM") as dr:

                def emit_quarter(g):
                    """Attention for q columns [1024*g, 1024*(g+1))."""
                    O_ps = po.tile([128, 2, 512], F32, tag="O", name="O_ps", bufs=1)
                    prev = None
                    for c in range(32):
                        s_t = psc.tile([128, 1024], F32, tag="s", name="s_t")
                        for j in range(2):
                            nc.tensor.matmul(s_t[:, ts(j, 512)], kT2[:, ts(c, 128)],
                                             qT2[:, ds(g * 1024 + j * 512, 512)],
                                             start=True, stop=True)
                        a_t = pa.tile([128, 1024], F32R, tag="a", name="a_t")
                        nc.scalar.activation(a_t[:], s_t[:], EXP, scale=0.125)
                        if prev is not None:
                            pc, pa_t = prev
                            for j in range(2):
                                nc.tensor.matmul(O_ps[:, j, :], va2[:, pc, :],
                                                 pa_t[:, ts(j, 512)],
                                                 start=(pc == 0), stop=(pc == 31))
                        prev = (c, a_t)
                    pc, pa_t = prev
                    for j in range(2):
                        nc.tensor.matmul(O_ps[:, j, :], va2[:, pc, :],
                                         pa_t[:, ts(j, 512)],
                                         start=(pc == 0), stop=(pc == 31))
                    # drain: full [128, 1024] copy (rows 64-127 = denominators)
                    nc.vector.tensor_copy(OT2[:, ts(g, 1024)], O_ps[:, :, :])
                    # denominators -> [128, 8] via DRAM roundtrip, reciprocal
                    scr = dr.tile([1, 1024], F32, tag="scr", name="scr")
                    nc.scalar.dma_start(scr[:], OT2[64:65, ts(g, 1024)].bitcast(F32))
                    nc.scalar.dma_start(
                        recipT[:, ts(g, 8)],
                        scr[0:1, :].rearrange("x (j p) -> (x p) j", p=128))
                    nc.vector.reciprocal(recipT[:, ts(g, 8)], recipT[:, ts(g, 8)])

                def emit_y(g):
                    """Output projection for q-quarter g (8 chunks of 128)."""
                    for i in range(8):
                        qc = g * 8 + i
                        p = po.tile([128, 512], F32, tag="y", name="p_y")
                        nc.tensor.matmul(p[:], OT2[:, ts(qc, 128)], wo2[:],
                                         start=True, stop=True)
                        y_t = yt.tile([128, 512], F32, tag="yt", name="y_t")
                        nc.vector.tensor_scalar_mul(y_t[:], p[:], recipT[:, qc:qc + 1])
                        nc.sync.dma_start(y_d[ts(qc, 128), :], y_t[:])

                for g in range(4):
                    emit_quarter(g)
                    if g > 0:
                        emit_y(g - 1)
                emit_y(3)

    nc.compile()
    return nc


def _host_prep(x, wq, wk, wv, wo, timestamp):
    x2 = np.asarray(x, dtype=np.float32).reshape(T, HIDDEN)
    xT = np.ascontiguousarray(x2.T)

    tsamp = np.asarray(timestamp).reshape(T)
    inv = (1.0 / (np.float32(ROPE_BASE)
                  ** (np.arange(0, HD, 2, dtype=np.float32) / np.float32(HD))))
    freqs = tsamp.astype(np.float32)[:, None] * inv[None, :].astype(np.float32)
    emb = np.concatenate([freqs, freqs], axis=1)
    cs = np.concatenate([np.cos(emb).T, np.sin(emb).T], axis=0)
    cs = np.ascontiguousarray(cs, dtype=np.float32)

    P = np.zeros((HD, HD), dtype=np.float32)
    P[np.arange(32), np.arange(32) + 32] = -1.0
    P[np.arange(32) + 32, np.arange(32)] = 1.0

    istk2 = np.zeros((128, 128), dtype=np.float32)
    istk2[0:64, 0:64] = np.eye(HD)
    istk2[64:128, 0:64] = np.eye(HD)
    iden = np.eye(HD, dtype=np.float32)

    wq = np.asarray(wq, dtype=np.float32)
    wk = np.asarray(wk, dtype=np.float32)
    wv = np.asarray(wv, dtype=np.float32)
    wo = np.asarray(wo, dtype=np.float32)

    in_maps = []
    for h in range(N_HEADS):
        sl = slice(h * HD, (h + 1) * HD)
        wq_h, wk_h, wv_h = wq[sl, :], wk[sl, :], wv[sl, :]
        wo2 = np.zeros((128, HIDDEN), dtype=np.float32)
        wo2[0:HD, :] = wo[:, sl].T
        in_maps.append({
            "xT": xT,
            "cs": cs,
            "wqcat": np.ascontiguousarray(
                np.concatenate([wq_h.T, (P @ wq_h).T], axis=1)),
            "wkcat": np.ascontiguousarray(
                np.concatenate([wk_h.T, (P @ wk_h).T], axis=1)),
            "wvT": np.ascontiguousarray(wv_h.T),
            "wo2": wo2,
            "istk2": istk2,
            "iden": iden,
            "ones": np.ones((128, 32, HD), dtype=np.float32),
        })
    return in_maps


def kernel(x, wq, wk, wv, wo, timestamp):
    if "nc" not in _CACHE:
        _CACHE["nc"] = _build()
    nc = _CACHE["nc"]
    in_maps = _host_prep(x, wq, wk, wv, wo, timestamp)
    r = run_bass_kernel_spmd(nc, in_maps, list(range(N_CORES)))
    y = np.zeros((T, HIDDEN), dtype=np.float64)
    for c in range(N_CORES):
        y += r.results[c]["y"].astype(np.float64)
    return y.astype(np.float32).reshape(1, T, HIDDEN)


# revision 8
# speedup vs baseline: 1.5845x; 1.0285x over previous
"""Trainium2 Bass kernel for nn_NeuralAttention (dense transformer block:
QKV projection + RoPE + softmax attention + output projection).

Sharding: 8 heads -> 8 NeuronCores (tensor parallel, Megatron-style).
Each core computes one head end-to-end from the full input x and produces a
partial output y_h = softmax((q_h k_h^T)/8) v_h @ wo[:, h].T of shape
[4096, 512]; the host sums the 8 partials.

All matmul operands are float32r (tf32-like, ~1e-4 relative error, 1 col/cycle
on the PE). Every contraction is padded to K=128: the TRN2 HAM clock gate
only counts the PE "busy" when the full array is active, so K=64 matmuls run
at 1.2 GHz forever while K=128 runs at 2.4 GHz. Padding the contraction with
zeros (producer side) or multiplying garbage rows by zero weights (consumer
side) doubles the clock at zero cycle cost.

Per-core pipeline (head h):
  xT [512, 4096] (host-transposed x) -> q|rot(q) and k|rot(k) concat-M
  projections -> RoPE via DVE mult with [cos;sin] + PE fold with
  [I;I | 0] -> q'T/k'T [128, 4096] (rows 64-127 zero)
  S.T chunk [128(tk), 512(q)] = kT2_chunk.T @ qT2 (K=128, hi half zeros)
  A.T = exp(S.T/8)  (ACT, PSUM->SBUF, f32r)
  O.T [128, q] += va2_chunk.T @ A.T  (va2 cols 64-127 = 1.0 -> rows 64-127 of
  O.T = softmax denominators), accumulated in PSUM over 32 chunks
  y qchunk [128, 512] = (OT2_chunk.T @ wo2) * recip(sums)  (wo2 rows 64-127
  zero), overlapped with the next q-quarter's attention.
"""

import numpy as np

import concourse.bacc as bacc
import concourse.tile as tile
from concourse import mybir
from concourse.bass import ds, ts
from concourse.bass_utils import run_bass_kernel_spmd

F32 = mybir.dt.float32
F32R = mybir.dt.float32r
EXP = mybir.ActivationFunctionType.Exp

T = 4096
HIDDEN = 512
N_HEADS = 8
HD = 64
N_CORES = 8
NBLK = T // 512
ROPE_BASE = 10000.0

_CACHE = {}


def _fold(nc, pf, istk2, qT2, kT2, pending):
    tq0, tk0, b0 = pending.pop(0)
    pfq = pf.tile([128, 512], F32, tag="pf", name="pfq")
    nc.tensor.matmul(pfq[:], istk2[:], tq0[:], start=True, stop=True)
    nc.scalar.copy(qT2[:, ts(b0, 512)], pfq[:].bitcast(F32))
    pfk = pf.tile([128, 512], F32, tag="pf", name="pfk")
    nc.tensor.matmul(pfk[:], istk2[:], tk0[:], start=True, stop=True)
    nc.scalar.copy(kT2[:, ts(b0, 512)], pfk[:].bitcast(F32))


def _build():
    nc = bacc.Bacc("TRN2", target_bir_lowering=False, debug=False,
                   num_devices=N_CORES)

    xT_d = nc.dram_tensor("xT", [4, NBLK, 128, 512], F32, kind="ExternalInput").ap()
    cs_d = nc.dram_tensor("cs", [128, T], F32, kind="ExternalInput").ap()
    wq_d = nc.dram_tensor("wqcat", [HIDDEN, 128], F32, kind="ExternalInput").ap()
    wk_d = nc.dram_tensor("wkcat", [HIDDEN, 128], F32, kind="ExternalInput").ap()
    wv_d = nc.dram_tensor("wvT", [HIDDEN, HD], F32, kind="ExternalInput").ap()
    wo_d = nc.dram_tensor("wo2", [128, HIDDEN], F32, kind="ExternalInput").ap()
    istk_d = nc.dram_tensor("istk2", [128, 128], F32, kind="ExternalInput").ap()
    iden_d = nc.dram_tensor("iden", [HD, HD], F32, kind="ExternalInput").ap()
    ones_d = nc.dram_tensor("ones", [128, 32, HD], F32, kind="ExternalInput").ap()
    y_d = nc.dram_tensor("y", [T, HIDDEN], F32, kind="ExternalOutput").ap()

    with tile.TileContext(nc) as tc:
        with tc.tile_pool(name="persist", bufs=1) as sb:
            xT = sb.tile([128, 4, T], F32R)        # 64 KB/part
            cs = sb.tile([128, T], F32)            # 16 KB/part
            wq = sb.tile([128, 4, 128], F32R)
            wk = sb.tile([128, 4, 128], F32R)
            wv = sb.tile([128, 4, HD], F32R)
            wo2 = sb.tile([128, HIDDEN], F32R)     # rows 64-127 zero
            istk2 = sb.tile([128, 128], F32R)      # [[I;I] | 0]
            iden = sb.tile([HD, HD], F32R)
            qT2 = sb.tile([128, T], F32R)          # rows 64-127 zero
            kT2 = sb.tile([128, T], F32R)          # rows 64-127 zero
            va2 = sb.tile([128, 32, 128], F32R)    # cols 64-127 = 1.0
            OT2 = sb.tile([128, T], F32R)          # rows 64-127 = denominators
            recipT = sb.tile([128, 32], F32)

            # input DMAs; xT pieces contiguous in DRAM, b-outer so the
            # projections can start after the first 1 MB; two HWDGE queues
            for b in range(NBLK):
                for k in range(4):
                    eng = nc.sync if (k % 2 == 0) else nc.scalar
                    eng.dma_start(xT[:, k, ts(b, 512)],
                                  xT_d[k, b, :, :].bitcast(F32R))
            nc.sync.dma_start(cs[:], cs_d)
            nc.sync.dma_start(wq[:], wq_d.rearrange("(c p) m -> p c m", p=128).bitcast(F32R))
            nc.sync.dma_start(wk[:], wk_d.rearrange("(c p) m -> p c m", p=128).bitcast(F32R))
            nc.sync.dma_start(wv[:], wv_d.rearrange("(c p) m -> p c m", p=128).bitcast(F32R))
            nc.sync.dma_start(wo2[:], wo_d.bitcast(F32R))
            nc.sync.dma_start(istk2[:], istk_d.bitcast(F32R))
            nc.sync.dma_start(iden[:], iden_d.bitcast(F32R))
            nc.gpsimd.dma_start(va2[:, :, HD:128], ones_d.bitcast(F32R))

            # ─── Phase P: projections + RoPE + v transposes ───
            with tc.tile_pool(name="pp", bufs=2, space="PSUM") as pp, \
                 tc.tile_pool(name="pf", bufs=2, space="PSUM") as pf, \
                 tc.tile_pool(name="pv", bufs=2, space="PSUM") as pv, \
                 tc.tile_pool(name="ptr", bufs=2, space="PSUM") as ptr, \
                 tc.tile_pool(name="ptmp", bufs=3) as ptmp:

                # warm the exp table set early (overlaps with projections)
                warm = ptmp.tile([1, 16], F32, tag="warm", bufs=1)
                nc.vector.memset(warm[:], 0.0)
                nc.scalar.activation(warm[:], warm[:], EXP, scale=1.0)

                vT = ptmp.tile([HD, T], F32R, tag="vT", bufs=1)
                pending = []
                for b in range(NBLK):
                    pq = pp.tile([128, 512], F32, tag="pp", name="pq")
                    for k in range(4):
                        nc.tensor.matmul(pq[:], wq[:, k, :], xT[:, k, ts(b, 512)],
                                         start=(k == 0), stop=(k == 3))
                    pk = pp.tile([128, 512], F32, tag="pp", name="pk")
                    for k in range(4):
                        nc.tensor.matmul(pk[:], wk[:, k, :], xT[:, k, ts(b, 512)],
                                         start=(k == 0), stop=(k == 3))
                    pv_ = pv.tile([HD, 512], F32, tag="pv", name="pv_")
                    for k in range(4):
                        nc.tensor.matmul(pv_[:], wv[:, k, :], xT[:, k, ts(b, 512)],
                                         start=(k == 0), stop=(k == 3))
                    nc.scalar.copy(vT[:, ts(b, 512)], pv_[:].bitcast(F32))

                    tq = ptmp.tile([128, 512], F32R, tag="tq", name="tq")
                    nc.vector.tensor_tensor(tq[:], pq[:], cs[:, ts(b, 512)],
                                            op=mybir.AluOpType.mult)
                    tk_ = ptmp.tile([128, 512], F32R, tag="tk", name="tk_")
                    nc.vector.tensor_tensor(tk_[:], pk[:], cs[:, ts(b, 512)],
                                            op=mybir.AluOpType.mult)
                    pending.append((tq, tk_, b))
                    if b > 0:
                        _fold(nc, pf, istk2, qT2, kT2, pending)
                _fold(nc, pf, istk2, qT2, kT2, pending)

                for c in range(32):
                    pt = ptr.tile([128, HD], F32R, tag="ptr", name="pt")
                    nc.tensor.transpose(pt[:], vT[:, ts(c, 128)], iden[:])
                    nc.vector.tensor_copy(va2[:, c, 0:HD], pt[:].bitcast(F32))

            # ─── Phase A+Y: attention quarters with overlapped output proj ───
            with tc.tile_pool(name="po", bufs=2, space="PSUM") as po, \
                 tc.tile_pool(name="psc", bufs=2, space="PSUM") as psc, \
                 tc.tile_pool(name="pa", bufs=3) as pa, \
                 tc.tile_pool(name="yt", bufs=4) as yt, \
                 tc.tile_pool(name="dr", bufs=4, space="DRAM") as dr:

                def emit_quarter(g):
                    """Attention for q columns [1024*g, 1024*(g+1))."""
                    O_ps = po.tile([128, 2, 512], F32, tag="O", name="O_ps", bufs=1)
                    prev = None
                    for c in range(32):
                        s_t = psc.tile([128, 1024], F32, tag="s", name="s_t")
                        for j in range(2):
                            nc.tensor.matmul(s_t[:, ts(j, 512)], kT2[:, ts(c, 128)],
                                             qT2[:, ds(g * 1024 + j * 512, 512)],
                                             start=True, stop=True)
                        a_t = pa.tile([128, 1024], F32R, tag="a", name="a_t")
                        nc.scalar.activation(a_t[:], s_t[:], EXP, scale=0.125)
                        if prev is not None:
                            pc, pa_t = prev
                            for j in range(2):
                                nc.tensor.matmul(O_ps[:, j, :], va2[:, pc, :],
                                                 pa_t[:, ts(j, 512)],
                                                 start=(pc == 0), stop=(pc == 31))
                        prev = (c, a_t)
                    pc, pa_t = prev
                    for j in range(2):
                        nc.tensor.matmul(O_ps[:, j, :], va2[:, pc, :],
                                         pa_t[:, ts(j, 512)],
                                         start=(pc == 0), stop=(pc == 31))
                    # drain: full [128, 1024] copy (rows 64-127 = denominators)
                    nc.vector.tensor_copy(OT2[:, ts(g, 1024)], O_ps[:, :, :])
                    # denominators -> [128, 8] via DRAM roundtrip, reciprocal
                    scr = dr.tile([1, 1024], F32, tag="scr", name="scr")
                    nc.sync.dma_start(scr[:], OT2[64:65, ts(g, 1024)].bitcast(F32))
                    nc.sync.dma_start(
                        recipT[:, ts(g, 8)],
                        scr[0:1, :].rearrange("x (j p) -> (x p) j", p=128))
                    nc.vector.reciprocal(recipT[:, ts(g, 8)], recipT[:, ts(g, 8)])

                def emit_y(g):
                    """Output projection for q-quarter g (8 chunks of 128)."""
                    for i in range(8):
                        qc = g * 8 + i
                        p = po.tile([128, 512], F32, tag="y", name="p_y")
                        nc.tensor.matmul(p[:], OT2[:, ts(qc, 128)], wo2[:],
                                         start=True, stop=True)
                        y_t = yt.tile([128, 512], F32, tag="yt", name="y_t")
                        nc.vector.tensor_scalar_mul(y_t[:], p[:], recipT[:, qc:qc + 1])
                        nc.sync.dma_start(y_d[ts(qc, 128), :], y_t[:])

                for g in range(4):
                    emit_quarter(g)
                    if g > 0:
                        emit_y(g - 1)
                emit_y(3)

    nc.compile()
    return nc


def _host_prep(x, wq, wk, wv, wo, timestamp):
    x2 = np.asarray(x, dtype=np.float32).reshape(T, HIDDEN)
    xT_full = x2.T  # [512, 4096]
    xT = np.ascontiguousarray(
        xT_full.reshape(4, 128, NBLK, 512).transpose(0, 2, 1, 3))

    tsamp = np.asarray(timestamp).reshape(T)
    inv = (1.0 / (np.float32(ROPE_BASE)
                  ** (np.arange(0, HD, 2, dtype=np.float32) / np.float32(HD))))
    freqs = tsamp.astype(np.float32)[:, None] * inv[None, :].astype(np.float32)
    emb = np.concatenate([freqs, freqs], axis=1)
    cs = np.concatenate([np.cos(emb).T, np.sin(emb).T], axis=0)
    cs = np.ascontiguousarray(cs, dtype=np.float32)

    P = np.zeros((HD, HD), dtype=np.float32)
    P[np.arange(32), np.arange(32) + 32] = -1.0
    P[np.arange(32) + 32, np.arange(32)] = 1.0

    istk2 = np.zeros((128, 128), dtype=np.float32)
    istk2[0:64, 0:64] = np.eye(HD)
    istk2[64:128, 0:64] = np.eye(HD)
    iden = np.eye(HD, dtype=np.float32)

    wq = np.asarray(wq, dtype=np.float32)
    wk = np.asarray(wk, dtype=np.float32)
    wv = np.asarray(wv, dtype=np.float32)
    wo = np.asarray(wo, dtype=np.float32)

    in_maps = []
    for h in range(N_HEADS):
        sl = slice(h * HD, (h + 1) * HD)
        wq_h, wk_h, wv_h = wq[sl, :], wk[sl, :], wv[sl, :]
        wo2 = np.zeros((128, HIDDEN), dtype=np.float32)
        wo2[0:HD, :] = wo[:, sl].T
        in_maps.append({
            "xT": xT,
            "cs": cs,
            "wqcat": np.ascontiguousarray(
                np.concatenate([wq_h.T, (P @ wq_h).T], axis=1)),
            "wkcat": np.ascontiguousarray(
                np.concatenate([wk_h.T, (P @ wk_h).T], axis=1)),
            "wvT": np.ascontiguousarray(wv_h.T),
            "wo2": wo2,
            "istk2": istk2,
            "iden": iden,
            "ones": np.ones((128, 32, HD), dtype=np.float32),
        })
    return in_maps


def kernel(x, wq, wk, wv, wo, timestamp):
    if "nc" not in _CACHE:
        _CACHE["nc"] = _build()
    nc = _CACHE["nc"]
    in_maps = _host_prep(x, wq, wk, wv, wo, timestamp)
    r = run_bass_kernel_spmd(nc, in_maps, list(range(N_CORES)))
    y = np.zeros((T, HIDDEN), dtype=np.float64)
    for c in range(N_CORES):
        y += r.results[c]["y"].astype(np.float64)
    return y.astype(np.float32).reshape(1, T, HIDDEN)
